# revision 23
# baseline (speedup 1.0000x reference)
"""BiLevelRoutingAttention (spiking) Trainium2 kernel, v2.

Sharding: one (t, b) pair per core (T=4 x B=2 = 8 cores). All windows of a
(t, b) live on one core, so the routed-KV gather is local. Routing (region
mean -> scores -> top-4) is computed on HOST in numpy (exact f32, identical
semantics to the reference) and passed per-core as a tiny selT [128, 32]
0/1 matrix -- no device collective needed.

Device pipeline per core:
  qkv:   x @ w -> PSUM; q spikes via scalar sigmoid(1e6*(x - th)) (saturates
         to exact 0/1); k/v spikes via vector is_ge vs f32 thresholds into
         persistent kvt tiles with preset ones columns
         [k(0:256) | v0(256:384) | 1 | v1(385:513) | 1].
  gram:  per (c, w): G = K_c^T [V_c | 1] in PSUM (2 matmuls of 129 cols,
         PSUM-accumulated over the two 128-token chunks), scalar-copy to
         bf16 into a persistent all-window SBUF tile gsb_all [p, w, c, e].
  combine: per p-quarter of the flat (p, c, e) gram rows: 4 SBUF->SBUF
         transposing DMAs gather grow [(j w), 2064]; sel^T block-diag
         matmuls in PE quadrants (tile_position); mask to block-diag
         (vector, PSUM source); 4 SBUF->SBUF transposing DMAs scatter into
         p-major kvread [p, w, c, e] (516B descriptor runs, overlapped).
  attn:  att = kv^T q, den = dexp^T q (dexp = ksum broadcast x bmask);
         epilogue: scalar copy+eps from PSUM, vector reciprocal + multiply;
         projection + bias -> bf16 output. (GpSimd has no PSUM port on
         TRN2, so all PSUM-reading elementwise ops sit on vector/scalar.)
"""

import numpy as np
import ml_dtypes

T, B, Lt, Lh, Lw, C = 4, 2, 8, 32, 32, 256
WT, WH, WW = 2, 4, 4
LT, LH, LW = Lt // WT, Lh // WH, Lw // WW  # 4, 8, 8
W = WT * WH * WW        # 32 windows
S = LT * LH * LW        # 256 tokens per window
NTOK = W * S            # 8192
H, D = 8, 32
TOPK = 4
NCORES = 8
CCH = 344               # combine N-chunk (6 per j-group per quarter)
BF16 = ml_dtypes.bfloat16

_CACHE = {}


def build_kernel(dbg=False):
    from concourse import bacc
    import concourse.mybir as mybir
    import concourse.tile as tile

    bf = mybir.dt.bfloat16
    f32 = mybir.dt.float32
    is_ge = mybir.AluOpType.is_ge
    mult = mybir.AluOpType.mult
    Act = mybir.ActivationFunctionType

    nc = bacc.Bacc("TRN2", target_bir_lowering=False, debug=False,
                   num_devices=NCORES)

    xT = nc.dram_tensor("xT", [2, 128, NTOK], bf, kind="ExternalInput")
    wq = nc.dram_tensor("wq", [128, 2, 2, 128], bf, kind="ExternalInput")
    wkv = nc.dram_tensor("wkv", [128, 2, 512], bf, kind="ExternalInput")
    thqs = nc.dram_tensor("thqs", [128, 2], f32, kind="ExternalInput")
    thkv = nc.dram_tensor("thkv", [128, 512], f32, kind="ExternalInput")
    wproj = nc.dram_tensor("wproj", [128, 2, 2, 128], bf, kind="ExternalInput")
    bproj = nc.dram_tensor("bproj", [128, 2], f32, kind="ExternalInput")
    bmask = nc.dram_tensor("bmask", [128, 128], bf, kind="ExternalInput")
    m64q = nc.dram_tensor("m64q", [128, 4, 2064], bf, kind="ExternalInput")
    selT4 = nc.dram_tensor("selT4", [128, 32], bf, kind="ExternalInput")
    outT = nc.dram_tensor("outT", [128, 2, NTOK], bf, kind="ExternalOutput")

    with tile.TileContext(nc) as tc:
        with (
            tc.tile_pool(name="big", bufs=2) as big_pool,
            tc.tile_pool(name="persist", bufs=1) as pp,
            tc.tile_pool(name="gsb", bufs=3) as gsb_pool,
            tc.tile_pool(name="grow", bufs=2) as grow_pool,
            tc.tile_pool(name="attn_sb", bufs=1) as asb_pool,
            tc.tile_pool(name="outp", bufs=4) as out_pool,
            tc.tile_pool(name="mm512", bufs=4, space="PSUM") as mm512,
            tc.tile_pool(name="attnp", bufs=4, space="PSUM") as attnp,
            tc.tile_pool(name="dram", bufs=1, space="DRAM") as dram_pool,
        ):
            # ---- weights first on the scalar queue (tiny, unblock compute) --
            wq_sb = pp.tile([128, 2, 2, 128], bf)
            nc.scalar.dma_start(wq_sb[:], wq[:])
            wkv_sb = pp.tile([128, 2, 512], bf)
            nc.scalar.dma_start(wkv_sb[:], wkv[:])
            thqs_sb = pp.tile([128, 2], f32)
            nc.scalar.dma_start(thqs_sb[:], thqs[:])
            thkv_sb = pp.tile([128, 512], f32)
            nc.scalar.dma_start(thkv_sb[:], thkv[:])
            wproj_sb = pp.tile([128, 2, 2, 128], bf)
            nc.scalar.dma_start(wproj_sb[:], wproj[:])
            bproj_sb = pp.tile([128, 2], f32)
            nc.scalar.dma_start(bproj_sb[:], bproj[:])
            bmask_sb = pp.tile([128, 128], bf)
            nc.scalar.dma_start(bmask_sb[:], bmask[:])
            m64_sb = pp.tile([128, 4, 2064], bf)
            nc.scalar.dma_start(m64_sb[:], m64q[:])
            selT_sb = pp.tile([128, 32], bf)
            nc.scalar.dma_start(selT_sb[:], selT4[:])

            # ---- x load (sync queue), first block early ----
            xsb = big_pool.tile([128, 2, NTOK], bf, tag="bigbuf", bufs=1)
            for (a, b2) in ((0, 1024), (1024, 4096), (4096, NTOK)):
                for c in range(2):
                    nc.sync.dma_start(xsb[:, c, a:b2], xT[c, :, a:b2])

            # ---- persistent kvt tiles with preset ones columns ----
            kvt = pp.tile([128, 12, 514], bf)
            nc.vector.memset(kvt[:, :, 384:385], 1.0)
            nc.vector.memset(kvt[:, :, 513:514], 1.0)
            eps_sb = pp.tile([128, 1], f32)
            nc.vector.memset(eps_sb[:], 1e-6)

            # ---- qkv + spikes + per-window Grams ----
            qsb = big_pool.tile([128, 2, NTOK], bf, tag="qsb", bufs=1)
            # gram_dram[w, p, c, e]: w-major rows, flat (p c e) per window
            gram_dram = dram_pool.tile([32, 128, 2, 129], bf)
            # kv + grams, software-pipelined: gram(N) is emitted after
            # kv(N+1) so the PE never stalls at a gram matmul waiting for
            # the same block's vector spikes
            def emit_kv(blk):
                for tci in range(4):
                    slot = (blk % 3) * 4 + tci
                    tcg = blk * 4 + tci
                    ksl = slice(tcg * 128, (tcg + 1) * 128)
                    kvp = mm512.tile([128, 512], f32, tag="mm512")
                    for c in range(2):
                        nc.tensor.matmul(kvp[:], xsb[:, c, ksl],
                                         wkv_sb[:, c, :],
                                         start=(c == 0), stop=(c == 1))
                    nc.vector.tensor_tensor(kvt[:, slot, 0:384], kvp[:, 0:384],
                                            thkv_sb[:, 0:384], op=is_ge)
                    nc.vector.tensor_tensor(kvt[:, slot, 385:513],
                                            kvp[:, 384:512],
                                            thkv_sb[:, 384:512], op=is_ge)

            def emit_gram(blk):
                gsb = gsb_pool.tile([128, 2, 2, 129], bf, tag="gsb")
                for wi in range(2):
                    s0 = (blk % 3) * 4 + 2 * wi
                    s1 = s0 + 1
                    for c in range(2):
                        gp = mm512.tile([128, 129], f32, tag="mm512")
                        ksl2 = slice(c * 128, (c + 1) * 128)
                        rsl = slice(256 + c * 129, 256 + c * 129 + 129)
                        nc.tensor.matmul(gp[:], kvt[:, s0, ksl2],
                                         kvt[:, s0, rsl],
                                         start=True, stop=False)
                        nc.tensor.matmul(gp[:], kvt[:, s1, ksl2],
                                         kvt[:, s1, rsl],
                                         start=False, stop=True)
                        nc.scalar.activation(gsb[:, wi, c, :], gp[:],
                                             Act.Copy)
                eng = nc.sync if blk % 2 == 0 else nc.gpsimd
                eng.dma_start(
                    gram_dram[2 * blk:2 * blk + 2].rearrange(
                        "w p c e -> p w (c e)"),
                    gsb[:].rearrange("p wi c e -> p wi (c e)"))

            emit_kv(0)
            for blk in range(1, 16):
                emit_kv(blk)
                emit_gram(blk - 1)
            emit_gram(15)

            # ---- combine: sel^T @ gram rows, quarter-pipelined; p-major
            # kvr_dram so the kvread loads are big contiguous descriptors.
            # q projections interleave with the combine's DMA-bound phase ----
            kvr_dram = dram_pool.tile([128, 32, 2, 129], bf)
            gflat = gram_dram[:].rearrange("w p c e -> w (p c e)")
            kvread = asb_pool.tile([128, 32, 2, 129], bf, tag="kvread", bufs=1)

            def emit_q(blk):
                tsl = slice(blk * 512, (blk + 1) * 512)
                for qc in range(2):
                    qp = mm512.tile([128, 512], f32, tag="mm512")
                    for c in range(2):
                        nc.tensor.matmul(qp[:], wq_sb[:, c, qc, :],
                                         xsb[:, c, tsl],
                                         start=(c == 0), stop=(c == 1))
                    nc.scalar.activation(qsb[:, qc, tsl], qp[:], Act.Sigmoid,
                                         bias=thqs_sb[:, qc:qc + 1], scale=1e6)

            def emit_combine_qtr(qtr):
                grow = grow_pool.tile([128, 2064], bf, tag="grow", bufs=2)
                for j in range(4):
                    jsl = slice(qtr * 8256 + j * 2064,
                                qtr * 8256 + (j + 1) * 2064)
                    eng = nc.gpsimd if j % 2 == 0 else nc.sync
                    eng.dma_start(grow[32 * j:32 * (j + 1), :],
                                  gflat[:, jsl])
                # block-diag mask applied to grow (commutes with the sel
                # matmul, which is per-column) -- runs on idle gpsimd
                nc.gpsimd.tensor_tensor(grow[:], grow[:], m64_sb[:, qtr, :],
                                        op=mult)
                kvout = grow_pool.tile([128, 2064], bf, tag="kvout", bufs=2)
                for ch in range(6):
                    csl = slice(ch * CCH, (ch + 1) * CCH)
                    cp = mm512.tile([128, CCH], f32, tag="mm512")
                    for j in range(4):
                        nc.tensor.matmul(cp[32 * j:32 * (j + 1), :],
                                         selT_sb[32 * j:32 * (j + 1), :],
                                         grow[32 * j:32 * (j + 1), csl],
                                         start=True, stop=True,
                                         tile_position=(32 * j, 32 * j))
                    nc.scalar.activation(kvout[:, csl], cp[:], Act.Copy)
                for j in range(4):
                    psl = slice(32 * qtr + 8 * j, 32 * qtr + 8 * j + 8)
                    eng = nc.gpsimd if j % 2 == 0 else nc.sync
                    eng.dma_start(
                        kvr_dram[psl].rearrange("p w c e -> w p (c e)"),
                        kvout[32 * j:32 * (j + 1), :].rearrange(
                            "w (p ce) -> w p ce", p=8))
                nc.gpsimd.dma_start(kvread[32 * qtr:32 * (qtr + 1)],
                                    kvr_dram[32 * qtr:32 * (qtr + 1)])

            for grp in range(4):
                for blk in range(4 * grp, 4 * grp + 4):
                    emit_q(blk)
                emit_combine_qtr(grp)

            # ---- den lhsT: ksum column replicated block-diagonally ----
            dexp = asb_pool.tile([128, 32, 2, 128], bf, tag="dexp", bufs=1)
            for c in range(2):
                nc.vector.tensor_tensor(
                    dexp[:, :, c, :],
                    kvread[:, :, c, 128:129].to_broadcast([128, 32, 128]),
                    bmask_sb[:, None, :].to_broadcast([128, 32, 128]),
                    op=mult)

            # ---- attention + den + divide; projection ----
            attn_nb = big_pool.tile([128, 2, NTOK], bf, tag="bigbuf", bufs=1)
            for blk in range(16):
                for wi in range(2):
                    w = blk * 2 + wi
                    wsl = slice(w * 256, (w + 1) * 256)
                    adpA = attnp.tile([128, 512], f32, tag="attn")
                    adpB = attnp.tile([128, 512], f32, tag="attn")
                    for c in range(2):
                        nc.tensor.matmul(adpA[:, 256 * c:256 * (c + 1)],
                                         kvread[:, w, c, 0:128],
                                         qsb[:, c, wsl],
                                         start=True, stop=True)
                        nc.tensor.matmul(adpB[:, 256 * c:256 * (c + 1)],
                                         dexp[:, w, c, :],
                                         qsb[:, c, wsl],
                                         start=True, stop=True)
                    den_sc = out_pool.tile([128, 512], f32, tag="densc",
                                           bufs=4)
                    nc.scalar.activation(den_sc[:], adpB[:], Act.Identity,
                                         bias=eps_sb[:, 0:1])
                    nc.vector.reciprocal_approx_fast(out=den_sc[:],
                                                     in_=den_sc[:])
                    nc.vector.tensor_tensor(
                        attn_nb[:, :, wsl],
                        adpA[:].rearrange("p (c s) -> p c s", c=2),
                        den_sc[:].rearrange("p (c s) -> p c s", c=2),
                        op=mult)
                tsl = slice(blk * 512, (blk + 1) * 512)
                osb = out_pool.tile([128, 2, 512], bf, tag="osb", bufs=3)
                for pc in range(2):
                    pjp = mm512.tile([128, 512], f32, tag="mm512")
                    for ec in range(2):
                        nc.tensor.matmul(pjp[:], wproj_sb[:, ec, pc, :],
                                         attn_nb[:, ec, tsl],
                                         start=(ec == 0), stop=(ec == 1))
                    nc.scalar.activation(osb[:, pc, :], pjp[:], Act.Identity,
                                         bias=bproj_sb[:, pc:pc + 1])
                eng = nc.sync if blk % 2 == 0 else nc.gpsimd
                eng.dma_start(outT[:, :, tsl], osb[:])

    nc.compile()
    return nc


def _prep_shared(w_qkv, b_qkv, w_proj, b_proj):
    wq_a = w_qkv[:, 0:256].reshape(2, 128, 2, 128).transpose(1, 0, 2, 3)
    wkv_a = w_qkv[:, 256:768].reshape(2, 128, 512).transpose(1, 0, 2)
    th = 2.0 - b_qkv
    thq_a = th[0:256].reshape(2, 128).T
    thqs_a = -1e6 * thq_a
    thkv_a = np.broadcast_to(th[256:768], (128, 512))
    wproj_a = w_proj.reshape(2, 128, 2, 128).transpose(1, 0, 2, 3)
    bproj_a = b_proj.reshape(2, 128).T
    i = np.arange(128)[:, None]
    j = np.arange(128)[None, :]
    bmask_a = (i // 32) == (j // 32)
    # m64q[32*j + w, qtr, cw]: block-diag mask of flat gram col
    # flat = qtr*8256 + j*2064 + cw;  p = flat//258, e = (flat%258)%129
    rr = np.arange(128)[:, None, None]
    qq = np.arange(4)[None, :, None]
    cw = np.arange(2064)[None, None, :]
    flat = qq * 8256 + (rr // 32) * 2064 + cw
    pp_ = flat // 258
    ee = (flat % 258) % 129
    mask64q_a = ((pp_ // 32) == (ee // 32)) | (ee == 128)
    return {
        "wq": np.ascontiguousarray(wq_a).astype(BF16),
        "wkv": np.ascontiguousarray(wkv_a).astype(BF16),
        "thqs": np.ascontiguousarray(thqs_a).astype(np.float32),
        "thkv": np.ascontiguousarray(thkv_a).astype(np.float32),
        "wproj": np.ascontiguousarray(wproj_a).astype(BF16),
        "bproj": np.ascontiguousarray(bproj_a).astype(np.float32),
        "bmask": bmask_a.astype(BF16),
        "m64q": mask64q_a.astype(BF16),
    }


def window_partition(x):
    """[T,B,Lt,Lh,Lw,C] -> [T,B,NTOK,C] with tokens in (w, s) order."""
    Tb, Bb = x.shape[0], x.shape[1]
    xw = x.reshape(Tb, Bb, WT, LT, WH, LH, WW, LW, C)
    xw = xw.transpose(0, 1, 2, 4, 6, 3, 5, 7, 8)
    return np.ascontiguousarray(xw).reshape(Tb, Bb, NTOK, C)


def window_reverse(o):
    """[NTOK, C] -> [Lt, Lh, Lw, C]."""
    o = o.reshape(WT, WH, WW, LT, LH, LW, C)
    o = o.transpose(0, 3, 1, 4, 2, 5, 6)
    return np.ascontiguousarray(o).reshape(Lt, Lh, Lw, C)


def _routing_selT4(xw):
    """Host-side routing: region mean -> scores -> top-4 -> selT, replicated
    4x along partitions for the PE-quadrant combine. xw: [T,B,NTOK,C] f32."""
    xw5 = xw.reshape(T, B, W, S, C)
    region = xw5.mean(axis=(0, 3))                       # [B, W, C]
    scale = np.float32(D ** -0.5)
    scores = np.einsum('bwc,bvc->bwv', region, region) * scale
    idx = np.argsort(-scores, axis=-1, kind='stable')[:, :, :TOPK]
    sel = np.zeros((B, W, W), np.float32)
    b_ix = np.arange(B)[:, None, None]
    w_ix = np.arange(W)[None, :, None]
    sel[b_ix, w_ix, idx] = 1.0
    selT = sel.transpose(0, 2, 1)                        # [B, ws, wd]
    return np.tile(selT, (1, 4, 1)).astype(BF16)         # [B, 128, 32]


def run_kernel_spmd(nc, in_maps, **kwargs):
    from concourse.bass_utils import run_bass_kernel_spmd
    return run_bass_kernel_spmd(nc, in_maps, core_ids=list(range(NCORES)),
                                **kwargs)


def make_in_maps(x, w_qkv, b_qkv, w_proj, b_proj):
    x = np.asarray(x, dtype=np.float32)
    shared = _prep_shared(np.asarray(w_qkv, dtype=np.float32),
                          np.asarray(b_qkv, dtype=np.float32),
                          np.asarray(w_proj, dtype=np.float32),
                          np.asarray(b_proj, dtype=np.float32))
    xw = window_partition(x)
    selT4 = _routing_selT4(xw)
    in_maps = []
    for core in range(NCORES):
        b, t = core // 4, core % 4
        xt = np.ascontiguousarray(xw[t, b].T).astype(BF16)  # [C, NTOK]
        in_maps.append({**shared, "xT": xt.reshape(2, 128, NTOK),
                        "selT4": np.ascontiguousarray(selT4[b])})
    return in_maps


def unpack_out(res):
    out = np.empty((T, B, Lt, Lh, Lw, C), dtype=np.float32)
    for core in range(NCORES):
        b, t = core // 4, core % 4
        oT = np.asarray(res.results[core]["outT"],
                        dtype=np.float32).transpose(1, 0, 2).reshape(256, NTOK)
        out[t, b] = window_reverse(np.ascontiguousarray(oT.T))
    return out


def kernel(x, w_qkv, b_qkv, w_proj, b_proj):
    if "nc" not in _CACHE:
        _CACHE["nc"] = build_kernel()
    nc = _CACHE["nc"]
    in_maps = make_in_maps(x, w_qkv, b_qkv, w_proj, b_proj)
    res = run_kernel_spmd(nc, in_maps)
    return unpack_out(res)


# revision 24
# speedup vs baseline: 1.1542x; 1.1542x over previous
"""BiLevelRoutingAttention (spiking) Trainium2 kernel, v2.

Sharding: one (t, b) pair per core (T=4 x B=2 = 8 cores). All windows of a
(t, b) live on one core, so the routed-KV gather is local. Routing (region
mean -> scores -> top-4) is computed on HOST in numpy (exact f32, identical
semantics to the reference) and passed per-core as a tiny selT [128, 32]
0/1 matrix -- no device collective needed.

Device pipeline per core:
  qkv:   x @ w -> PSUM; q spikes via scalar sigmoid(1e6*(x - th)) (saturates
         to exact 0/1); k/v spikes via vector is_ge vs f32 thresholds into
         persistent kvt tiles with preset ones columns
         [k(0:256) | v0(256:384) | 1 | v1(385:513) | 1].
  gram:  per (c, w): G = K_c^T [V_c | 1] in PSUM (2 matmuls of 129 cols,
         PSUM-accumulated over the two 128-token chunks), scalar-copy to
         bf16 into a persistent all-window SBUF tile gsb_all [p, w, c, e].
  combine: per p-quarter of the flat (p, c, e) gram rows: 4 SBUF->SBUF
         transposing DMAs gather grow [(j w), 2064]; sel^T block-diag
         matmuls in PE quadrants (tile_position); mask to block-diag
         (vector, PSUM source); 4 SBUF->SBUF transposing DMAs scatter into
         p-major kvread [p, w, c, e] (516B descriptor runs, overlapped).
  attn:  att = kv^T q, den = dexp^T q (dexp = ksum broadcast x bmask);
         epilogue: scalar copy+eps from PSUM, vector reciprocal + multiply;
         projection + bias -> bf16 output. (GpSimd has no PSUM port on
         TRN2, so all PSUM-reading elementwise ops sit on vector/scalar.)
"""

import numpy as np
import ml_dtypes

T, B, Lt, Lh, Lw, C = 4, 2, 8, 32, 32, 256
WT, WH, WW = 2, 4, 4
LT, LH, LW = Lt // WT, Lh // WH, Lw // WW  # 4, 8, 8
W = WT * WH * WW        # 32 windows
S = LT * LH * LW        # 256 tokens per window
NTOK = W * S            # 8192
H, D = 8, 32
TOPK = 4
NCORES = 8
CCH = 344               # combine N-chunk (6 per j-group per quarter)
BF16 = ml_dtypes.bfloat16

_CACHE = {}


def build_kernel(dbg=False):
    from concourse import bacc
    import concourse.mybir as mybir
    import concourse.tile as tile

    bf = mybir.dt.bfloat16
    f32 = mybir.dt.float32
    is_ge = mybir.AluOpType.is_ge
    mult = mybir.AluOpType.mult
    Act = mybir.ActivationFunctionType

    nc = bacc.Bacc("TRN2", target_bir_lowering=False, debug=False,
                   num_devices=NCORES)

    xT = nc.dram_tensor("xT", [2, 128, NTOK], bf, kind="ExternalInput")
    wq = nc.dram_tensor("wq", [128, 2, 2, 128], bf, kind="ExternalInput")
    wkv = nc.dram_tensor("wkv", [128, 2, 512], bf, kind="ExternalInput")
    thqs = nc.dram_tensor("thqs", [128, 2], f32, kind="ExternalInput")
    thkv = nc.dram_tensor("thkv", [128, 512], f32, kind="ExternalInput")
    wproj = nc.dram_tensor("wproj", [128, 2, 2, 128], bf, kind="ExternalInput")
    bproj = nc.dram_tensor("bproj", [128, 2], f32, kind="ExternalInput")
    bmask = nc.dram_tensor("bmask", [128, 128], bf, kind="ExternalInput")
    m64q = nc.dram_tensor("m64q", [128, 4, 2064], bf, kind="ExternalInput")
    selT4 = nc.dram_tensor("selT4", [128, 32], bf, kind="ExternalInput")
    outT = nc.dram_tensor("outT", [128, 2, NTOK], bf, kind="ExternalOutput")

    with tile.TileContext(nc) as tc:
        with (
            tc.tile_pool(name="big", bufs=2) as big_pool,
            tc.tile_pool(name="persist", bufs=1) as pp,
            tc.tile_pool(name="gsb", bufs=3) as gsb_pool,
            tc.tile_pool(name="grow", bufs=2) as grow_pool,
            tc.tile_pool(name="attn_sb", bufs=1) as asb_pool,
            tc.tile_pool(name="outp", bufs=4) as out_pool,
            tc.tile_pool(name="mm512", bufs=4, space="PSUM") as mm512,
            tc.tile_pool(name="attnp", bufs=4, space="PSUM") as attnp,
            tc.tile_pool(name="dram", bufs=1, space="DRAM") as dram_pool,
        ):
            # ---- weights first on the scalar queue (tiny, unblock compute) --
            wq_sb = pp.tile([128, 2, 2, 128], bf)
            nc.scalar.dma_start(wq_sb[:], wq[:])
            wkv_sb = pp.tile([128, 2, 512], bf)
            nc.scalar.dma_start(wkv_sb[:], wkv[:])
            thqs_sb = pp.tile([128, 2], f32)
            nc.scalar.dma_start(thqs_sb[:], thqs[:])
            thkv_sb = pp.tile([128, 512], f32)
            nc.scalar.dma_start(thkv_sb[:], thkv[:])
            wproj_sb = pp.tile([128, 2, 2, 128], bf)
            nc.scalar.dma_start(wproj_sb[:], wproj[:])
            bproj_sb = pp.tile([128, 2], f32)
            nc.scalar.dma_start(bproj_sb[:], bproj[:])
            bmask_sb = pp.tile([128, 128], bf)
            nc.scalar.dma_start(bmask_sb[:], bmask[:])
            m64_sb = pp.tile([128, 4, 2064], bf)
            nc.scalar.dma_start(m64_sb[:], m64q[:])
            selT_sb = pp.tile([128, 32], bf)
            nc.scalar.dma_start(selT_sb[:], selT4[:])

            # ---- x load (sync queue), first block early ----
            xsb = big_pool.tile([128, 2, NTOK], bf, tag="bigbuf", bufs=1)
            for (a, b2) in ((0, 1024), (1024, 4096), (4096, NTOK)):
                for c in range(2):
                    nc.sync.dma_start(xsb[:, c, a:b2], xT[c, :, a:b2])

            # ---- persistent kvt tiles with preset ones columns ----
            kvt = pp.tile([128, 12, 514], bf)
            nc.vector.memset(kvt[:, :, 384:385], 1.0)
            nc.vector.memset(kvt[:, :, 513:514], 1.0)
            eps_sb = pp.tile([128, 1], f32)
            nc.vector.memset(eps_sb[:], 1e-6)

            # ---- qkv + spikes + per-window Grams ----
            qsb = big_pool.tile([128, 2, NTOK], bf, tag="qsb", bufs=1)
            # gram_dram[w, p, c, e]: w-major rows, flat (p c e) per window
            gram_dram = dram_pool.tile([32, 128, 2, 129], bf)
            # kv + grams, software-pipelined: gram(N) is emitted after
            # kv(N+1) so the PE never stalls at a gram matmul waiting for
            # the same block's vector spikes
            def emit_kv(blk):
                for tci in range(4):
                    slot = (blk % 3) * 4 + tci
                    tcg = blk * 4 + tci
                    ksl = slice(tcg * 128, (tcg + 1) * 128)
                    kvp = mm512.tile([128, 512], f32, tag="mm512")
                    for c in range(2):
                        nc.tensor.matmul(kvp[:], xsb[:, c, ksl],
                                         wkv_sb[:, c, :],
                                         start=(c == 0), stop=(c == 1))
                    nc.vector.tensor_tensor(kvt[:, slot, 0:384], kvp[:, 0:384],
                                            thkv_sb[:, 0:384], op=is_ge)
                    nc.vector.tensor_tensor(kvt[:, slot, 385:513],
                                            kvp[:, 384:512],
                                            thkv_sb[:, 384:512], op=is_ge)

            def emit_gram(blk):
                gsb = gsb_pool.tile([128, 2, 2, 129], bf, tag="gsb")
                for wi in range(2):
                    s0 = (blk % 3) * 4 + 2 * wi
                    s1 = s0 + 1
                    for c in range(2):
                        gp = mm512.tile([128, 129], f32, tag="mm512")
                        ksl2 = slice(c * 128, (c + 1) * 128)
                        rsl = slice(256 + c * 129, 256 + c * 129 + 129)
                        nc.tensor.matmul(gp[:], kvt[:, s0, ksl2],
                                         kvt[:, s0, rsl],
                                         start=True, stop=False)
                        nc.tensor.matmul(gp[:], kvt[:, s1, ksl2],
                                         kvt[:, s1, rsl],
                                         start=False, stop=True)
                        nc.scalar.activation(gsb[:, wi, c, :], gp[:],
                                             Act.Copy)
                eng = nc.sync if blk % 2 == 0 else nc.gpsimd
                eng.dma_start(
                    gram_dram[2 * blk:2 * blk + 2].rearrange(
                        "w p c e -> p w (c e)"),
                    gsb[:].rearrange("p wi c e -> p wi (c e)"))

            emit_kv(0)
            for blk in range(1, 16):
                emit_kv(blk)
                emit_gram(blk - 1)
            emit_gram(15)

            # ---- combine: sel^T @ gram rows, quarter-pipelined; p-major
            # kvr_dram so the kvread loads are big contiguous descriptors.
            # q projections interleave with the combine's DMA-bound phase ----
            kvr_dram = dram_pool.tile([128, 32, 2, 129], bf)
            gflat = gram_dram[:].rearrange("w p c e -> w (p c e)")
            kvread = asb_pool.tile([128, 32, 2, 129], bf, tag="kvread", bufs=1)

            def emit_q(blk):
                tsl = slice(blk * 512, (blk + 1) * 512)
                for qc in range(2):
                    qp = mm512.tile([128, 512], f32, tag="mm512")
                    for c in range(2):
                        nc.tensor.matmul(qp[:], wq_sb[:, c, qc, :],
                                         xsb[:, c, tsl],
                                         start=(c == 0), stop=(c == 1))
                    nc.scalar.activation(qsb[:, qc, tsl], qp[:], Act.Sigmoid,
                                         bias=thqs_sb[:, qc:qc + 1], scale=1e6)

            def emit_combine_qtr(qtr):
                grow = grow_pool.tile([128, 2064], bf, tag="grow", bufs=2)
                for j in range(4):
                    jsl = slice(qtr * 8256 + j * 2064,
                                qtr * 8256 + (j + 1) * 2064)
                    eng = nc.gpsimd if j % 2 == 0 else nc.sync
                    eng.dma_start(grow[32 * j:32 * (j + 1), :],
                                  gflat[:, jsl])
                # block-diag mask applied to grow (commutes with the sel
                # matmul, which is per-column); bf16 SBUF = 2x DVE mode
                nc.vector.tensor_tensor(grow[:], grow[:], m64_sb[:, qtr, :],
                                        op=mult)
                kvout = grow_pool.tile([128, 2064], bf, tag="kvout", bufs=2)
                for ch in range(6):
                    csl = slice(ch * CCH, (ch + 1) * CCH)
                    cp = mm512.tile([128, CCH], f32, tag="mm512")
                    for j in range(4):
                        nc.tensor.matmul(cp[32 * j:32 * (j + 1), :],
                                         selT_sb[32 * j:32 * (j + 1), :],
                                         grow[32 * j:32 * (j + 1), csl],
                                         start=True, stop=True,
                                         tile_position=(32 * j, 32 * j))
                    nc.scalar.activation(kvout[:, csl], cp[:], Act.Copy)
                for j in range(4):
                    psl = slice(32 * qtr + 8 * j, 32 * qtr + 8 * j + 8)
                    eng = nc.gpsimd if j % 2 == 0 else nc.sync
                    eng.dma_start(
                        kvr_dram[psl].rearrange("p w c e -> w p (c e)"),
                        kvout[32 * j:32 * (j + 1), :].rearrange(
                            "w (p ce) -> w p ce", p=8))
                nc.gpsimd.dma_start(kvread[32 * qtr:32 * (qtr + 1)],
                                    kvr_dram[32 * qtr:32 * (qtr + 1)])

            for grp in range(4):
                for blk in range(4 * grp, 4 * grp + 4):
                    emit_q(blk)
                emit_combine_qtr(grp)

            # ---- den lhsT: ksum column replicated block-diagonally ----
            dexp = asb_pool.tile([128, 32, 2, 128], bf, tag="dexp", bufs=1)
            for c in range(2):
                nc.vector.tensor_tensor(
                    dexp[:, :, c, :],
                    kvread[:, :, c, 128:129].to_broadcast([128, 32, 128]),
                    bmask_sb[:, None, :].to_broadcast([128, 32, 128]),
                    op=mult)

            # ---- attention + den + divide; projection ----
            attn_nb = big_pool.tile([128, 2, NTOK], bf, tag="bigbuf", bufs=1)
            for blk in range(16):
                for wi in range(2):
                    w = blk * 2 + wi
                    wsl = slice(w * 256, (w + 1) * 256)
                    adpA = attnp.tile([128, 512], f32, tag="attn")
                    adpB = attnp.tile([128, 512], f32, tag="attn")
                    for c in range(2):
                        nc.tensor.matmul(adpA[:, 256 * c:256 * (c + 1)],
                                         kvread[:, w, c, 0:128],
                                         qsb[:, c, wsl],
                                         start=True, stop=True)
                        nc.tensor.matmul(adpB[:, 256 * c:256 * (c + 1)],
                                         dexp[:, w, c, :],
                                         qsb[:, c, wsl],
                                         start=True, stop=True)
                    den_sc = out_pool.tile([128, 512], f32, tag="densc",
                                           bufs=4)
                    nc.scalar.activation(den_sc[:], adpB[:], Act.Identity,
                                         bias=eps_sb[:, 0:1])
                    nc.vector.reciprocal_approx_fast(out=den_sc[:],
                                                     in_=den_sc[:])
                    nc.vector.tensor_tensor(
                        attn_nb[:, :, wsl],
                        adpA[:].rearrange("p (c s) -> p c s", c=2),
                        den_sc[:].rearrange("p (c s) -> p c s", c=2),
                        op=mult)
                tsl = slice(blk * 512, (blk + 1) * 512)
                osb = out_pool.tile([128, 2, 512], bf, tag="osb", bufs=3)
                for pc in range(2):
                    pjp = mm512.tile([128, 512], f32, tag="mm512")
                    for ec in range(2):
                        nc.tensor.matmul(pjp[:], wproj_sb[:, ec, pc, :],
                                         attn_nb[:, ec, tsl],
                                         start=(ec == 0), stop=(ec == 1))
                    nc.scalar.activation(osb[:, pc, :], pjp[:], Act.Identity,
                                         bias=bproj_sb[:, pc:pc + 1])
                eng = nc.sync if blk % 2 == 0 else nc.gpsimd
                eng.dma_start(outT[:, :, tsl], osb[:])

    nc.compile()
    return nc


def _prep_shared(w_qkv, b_qkv, w_proj, b_proj):
    wq_a = w_qkv[:, 0:256].reshape(2, 128, 2, 128).transpose(1, 0, 2, 3)
    wkv_a = w_qkv[:, 256:768].reshape(2, 128, 512).transpose(1, 0, 2)
    th = 2.0 - b_qkv
    thq_a = th[0:256].reshape(2, 128).T
    thqs_a = -1e6 * thq_a
    thkv_a = np.broadcast_to(th[256:768], (128, 512))
    wproj_a = w_proj.reshape(2, 128, 2, 128).transpose(1, 0, 2, 3)
    bproj_a = b_proj.reshape(2, 128).T
    i = np.arange(128)[:, None]
    j = np.arange(128)[None, :]
    bmask_a = (i // 32) == (j // 32)
    # m64q[32*j + w, qtr, cw]: block-diag mask of flat gram col
    # flat = qtr*8256 + j*2064 + cw;  p = flat//258, e = (flat%258)%129
    rr = np.arange(128)[:, None, None]
    qq = np.arange(4)[None, :, None]
    cw = np.arange(2064)[None, None, :]
    flat = qq * 8256 + (rr // 32) * 2064 + cw
    pp_ = flat // 258
    ee = (flat % 258) % 129
    mask64q_a = ((pp_ // 32) == (ee // 32)) | (ee == 128)
    return {
        "wq": np.ascontiguousarray(wq_a).astype(BF16),
        "wkv": np.ascontiguousarray(wkv_a).astype(BF16),
        "thqs": np.ascontiguousarray(thqs_a).astype(np.float32),
        "thkv": np.ascontiguousarray(thkv_a).astype(np.float32),
        "wproj": np.ascontiguousarray(wproj_a).astype(BF16),
        "bproj": np.ascontiguousarray(bproj_a).astype(np.float32),
        "bmask": bmask_a.astype(BF16),
        "m64q": mask64q_a.astype(BF16),
    }


def window_partition(x):
    """[T,B,Lt,Lh,Lw,C] -> [T,B,NTOK,C] with tokens in (w, s) order."""
    Tb, Bb = x.shape[0], x.shape[1]
    xw = x.reshape(Tb, Bb, WT, LT, WH, LH, WW, LW, C)
    xw = xw.transpose(0, 1, 2, 4, 6, 3, 5, 7, 8)
    return np.ascontiguousarray(xw).reshape(Tb, Bb, NTOK, C)


def window_reverse(o):
    """[NTOK, C] -> [Lt, Lh, Lw, C]."""
    o = o.reshape(WT, WH, WW, LT, LH, LW, C)
    o = o.transpose(0, 3, 1, 4, 2, 5, 6)
    return np.ascontiguousarray(o).reshape(Lt, Lh, Lw, C)


def _routing_selT4(xw):
    """Host-side routing: region mean -> scores -> top-4 -> selT, replicated
    4x along partitions for the PE-quadrant combine. xw: [T,B,NTOK,C] f32."""
    xw5 = xw.reshape(T, B, W, S, C)
    region = xw5.mean(axis=(0, 3))                       # [B, W, C]
    scale = np.float32(D ** -0.5)
    scores = np.einsum('bwc,bvc->bwv', region, region) * scale
    idx = np.argsort(-scores, axis=-1, kind='stable')[:, :, :TOPK]
    sel = np.zeros((B, W, W), np.float32)
    b_ix = np.arange(B)[:, None, None]
    w_ix = np.arange(W)[None, :, None]
    sel[b_ix, w_ix, idx] = 1.0
    selT = sel.transpose(0, 2, 1)                        # [B, ws, wd]
    return np.tile(selT, (1, 4, 1)).astype(BF16)         # [B, 128, 32]


def run_kernel_spmd(nc, in_maps, **kwargs):
    from concourse.bass_utils import run_bass_kernel_spmd
    return run_bass_kernel_spmd(nc, in_maps, core_ids=list(range(NCORES)),
                                **kwargs)


def make_in_maps(x, w_qkv, b_qkv, w_proj, b_proj):
    x = np.asarray(x, dtype=np.float32)
    shared = _prep_shared(np.asarray(w_qkv, dtype=np.float32),
                          np.asarray(b_qkv, dtype=np.float32),
                          np.asarray(w_proj, dtype=np.float32),
                          np.asarray(b_proj, dtype=np.float32))
    xw = window_partition(x)
    selT4 = _routing_selT4(xw)
    in_maps = []
    for core in range(NCORES):
        b, t = core // 4, core % 4
        xt = np.ascontiguousarray(xw[t, b].T).astype(BF16)  # [C, NTOK]
        in_maps.append({**shared, "xT": xt.reshape(2, 128, NTOK),
                        "selT4": np.ascontiguousarray(selT4[b])})
    return in_maps


def unpack_out(res):
    out = np.empty((T, B, Lt, Lh, Lw, C), dtype=np.float32)
    for core in range(NCORES):
        b, t = core // 4, core % 4
        oT = np.asarray(res.results[core]["outT"],
                        dtype=np.float32).transpose(1, 0, 2).reshape(256, NTOK)
        out[t, b] = window_reverse(np.ascontiguousarray(oT.T))
    return out


def kernel(x, w_qkv, b_qkv, w_proj, b_proj):
    if "nc" not in _CACHE:
        _CACHE["nc"] = build_kernel()
    nc = _CACHE["nc"]
    in_maps = make_in_maps(x, w_qkv, b_qkv, w_proj, b_proj)
    res = run_kernel_spmd(nc, in_maps)
    return unpack_out(res)


# revision 29
# speedup vs baseline: 1.2834x; 1.1119x over previous
"""BiLevelRoutingAttention (spiking) Trainium2 kernel, v2.

Sharding: one (t, b) pair per core (T=4 x B=2 = 8 cores). All windows of a
(t, b) live on one core, so the routed-KV gather is local. Routing (region
mean -> scores -> top-4) is computed on HOST in numpy (exact f32, identical
semantics to the reference) and passed per-core as a tiny selT [128, 32]
0/1 matrix -- no device collective needed.

Device pipeline per core:
  qkv:   x @ w -> PSUM; q spikes via scalar sigmoid(1e6*(x - th)) (saturates
         to exact 0/1); k/v spikes via vector is_ge vs f32 thresholds into
         persistent kvt tiles with preset ones columns
         [k(0:256) | v0(256:384) | 1 | v1(385:513) | 1].
  gram:  per (c, w): G = K_c^T [V_c | 1] in PSUM (2 matmuls of 129 cols,
         PSUM-accumulated over the two 128-token chunks), scalar-copy to
         bf16 into a persistent all-window SBUF tile gsb_all [p, w, c, e].
  combine: per p-quarter of the flat (p, c, e) gram rows: 4 SBUF->SBUF
         transposing DMAs gather grow [(j w), 2064]; sel^T block-diag
         matmuls in PE quadrants (tile_position); mask to block-diag
         (vector, PSUM source); 4 SBUF->SBUF transposing DMAs scatter into
         p-major kvread [p, w, c, e] (516B descriptor runs, overlapped).
  attn:  att = kv^T q, den = dexp^T q (dexp = ksum broadcast x bmask);
         epilogue: scalar copy+eps from PSUM, vector reciprocal + multiply;
         projection + bias -> bf16 output. (GpSimd has no PSUM port on
         TRN2, so all PSUM-reading elementwise ops sit on vector/scalar.)
"""

import numpy as np
import ml_dtypes

T, B, Lt, Lh, Lw, C = 4, 2, 8, 32, 32, 256
WT, WH, WW = 2, 4, 4
LT, LH, LW = Lt // WT, Lh // WH, Lw // WW  # 4, 8, 8
W = WT * WH * WW        # 32 windows
S = LT * LH * LW        # 256 tokens per window
NTOK = W * S            # 8192
H, D = 8, 32
TOPK = 4
NCORES = 8
CCH = 344               # combine N-chunk (6 per j-group per quarter)
BF16 = ml_dtypes.bfloat16

_CACHE = {}


def build_kernel(dbg=False):
    from concourse import bacc
    import concourse.mybir as mybir
    import concourse.tile as tile

    bf = mybir.dt.bfloat16
    f32 = mybir.dt.float32
    is_ge = mybir.AluOpType.is_ge
    mult = mybir.AluOpType.mult
    Act = mybir.ActivationFunctionType

    nc = bacc.Bacc("TRN2", target_bir_lowering=False, debug=False,
                   num_devices=NCORES)

    xT = nc.dram_tensor("xT", [2, 128, NTOK], bf, kind="ExternalInput")
    wq = nc.dram_tensor("wq", [128, 2, 2, 128], bf, kind="ExternalInput")
    wkv = nc.dram_tensor("wkv", [128, 2, 512], bf, kind="ExternalInput")
    thqs = nc.dram_tensor("thqs", [128, 2], f32, kind="ExternalInput")
    thkv = nc.dram_tensor("thkv", [128, 512], f32, kind="ExternalInput")
    wproj = nc.dram_tensor("wproj", [128, 2, 2, 128], bf, kind="ExternalInput")
    bproj = nc.dram_tensor("bproj", [128, 2], f32, kind="ExternalInput")
    bmask = nc.dram_tensor("bmask", [128, 128], bf, kind="ExternalInput")
    m64q = nc.dram_tensor("m64q", [128, 4, 2064], bf, kind="ExternalInput")
    selT4 = nc.dram_tensor("selT4", [128, 32], bf, kind="ExternalInput")
    outT = nc.dram_tensor("outT", [128, 2, NTOK], bf, kind="ExternalOutput")

    with tile.TileContext(nc) as tc:
        with (
            tc.tile_pool(name="big", bufs=2) as big_pool,
            tc.tile_pool(name="persist", bufs=1) as pp,
            tc.tile_pool(name="gsb", bufs=3) as gsb_pool,
            tc.tile_pool(name="grow", bufs=2) as grow_pool,
            tc.tile_pool(name="attn_sb", bufs=1) as asb_pool,
            tc.tile_pool(name="outp", bufs=4) as out_pool,
            tc.tile_pool(name="mm512", bufs=4, space="PSUM") as mm512,
            tc.tile_pool(name="attnp", bufs=4, space="PSUM") as attnp,
            tc.tile_pool(name="dram", bufs=1, space="DRAM") as dram_pool,
        ):
            # ---- weights first on the scalar queue (tiny, unblock compute) --
            wq_sb = pp.tile([128, 2, 2, 128], bf)
            nc.scalar.dma_start(wq_sb[:], wq[:])
            wkv_sb = pp.tile([128, 2, 512], bf)
            nc.scalar.dma_start(wkv_sb[:], wkv[:])
            thqs_sb = pp.tile([128, 2], f32)
            nc.scalar.dma_start(thqs_sb[:], thqs[:])
            thkv_sb = pp.tile([128, 512], f32)
            nc.scalar.dma_start(thkv_sb[:], thkv[:])
            wproj_sb = pp.tile([128, 2, 2, 128], bf)
            nc.scalar.dma_start(wproj_sb[:], wproj[:])
            bproj_sb = pp.tile([128, 2], f32)
            nc.scalar.dma_start(bproj_sb[:], bproj[:])
            bmask_sb = pp.tile([128, 128], bf)
            nc.scalar.dma_start(bmask_sb[:], bmask[:])
            m64_sb = pp.tile([128, 4, 2064], bf)
            nc.scalar.dma_start(m64_sb[:], m64q[:])
            selT_sb = pp.tile([128, 32], bf)
            nc.scalar.dma_start(selT_sb[:], selT4[:])

            # ---- x load (sync queue), first block early ----
            xsb = big_pool.tile([128, 2, NTOK], bf, tag="bigbuf", bufs=1)
            for (a, b2) in ((0, 1024), (1024, 4096), (4096, NTOK)):
                for c in range(2):
                    nc.sync.dma_start(xsb[:, c, a:b2], xT[c, :, a:b2])

            # ---- persistent kvt tiles with preset ones columns ----
            kvt = pp.tile([128, 12, 514], bf)
            nc.vector.memset(kvt[:, :, 384:385], 1.0)
            nc.vector.memset(kvt[:, :, 513:514], 1.0)
            eps_sb = pp.tile([128, 1], f32)
            nc.vector.memset(eps_sb[:], 1e-6)
            neg1e6_sb = pp.tile([128, 1], f32)
            nc.vector.memset(neg1e6_sb[:], -1e6)

            # ---- qkv + spikes + per-window Grams ----
            qsb = big_pool.tile([128, 2, NTOK], bf, tag="qsb", bufs=1)
            # gram_dram[w, p, c, e]: w-major rows, flat (p c e) per window
            gram_dram = dram_pool.tile([32, 128, 2, 129], bf)
            # kv + grams, software-pipelined: gram(N) is emitted after
            # kv(N+1) so the PE never stalls at a gram matmul waiting for
            # the same block's vector spikes
            def emit_kv(blk):
                for tci in range(4):
                    slot = (blk % 3) * 4 + tci
                    tcg = blk * 4 + tci
                    ksl = slice(tcg * 128, (tcg + 1) * 128)
                    kvp = mm512.tile([128, 512], f32, tag="mm512")
                    for c in range(2):
                        nc.tensor.matmul(kvp[:], xsb[:, c, ksl],
                                         wkv_sb[:, c, :],
                                         start=(c == 0), stop=(c == 1))
                    if tci % 2 == 0:
                        # wkv is pre-divided by th (host), so the spike is
                        # (x@w' >= 1): saturated sigmoid on the scalar engine
                        nc.scalar.activation(kvt[:, slot, 0:384],
                                             kvp[:, 0:384], Act.Sigmoid,
                                             bias=neg1e6_sb[:, 0:1],
                                             scale=1e6)
                        nc.scalar.activation(kvt[:, slot, 385:513],
                                             kvp[:, 384:512], Act.Sigmoid,
                                             bias=neg1e6_sb[:, 0:1],
                                             scale=1e6)
                    else:
                        nc.vector.tensor_tensor(kvt[:, slot, 0:384],
                                                kvp[:, 0:384],
                                                thkv_sb[:, 0:384], op=is_ge)
                        nc.vector.tensor_tensor(kvt[:, slot, 385:513],
                                                kvp[:, 384:512],
                                                thkv_sb[:, 384:512], op=is_ge)

            def emit_gram(blk):
                gsb = gsb_pool.tile([128, 2, 2, 129], bf, tag="gsb")
                for wi in range(2):
                    s0 = (blk % 3) * 4 + 2 * wi
                    s1 = s0 + 1
                    for c in range(2):
                        gp = mm512.tile([128, 129], f32, tag="mm512")
                        ksl2 = slice(c * 128, (c + 1) * 128)
                        rsl = slice(256 + c * 129, 256 + c * 129 + 129)
                        nc.tensor.matmul(gp[:], kvt[:, s0, ksl2],
                                         kvt[:, s0, rsl],
                                         start=True, stop=False)
                        nc.tensor.matmul(gp[:], kvt[:, s1, ksl2],
                                         kvt[:, s1, rsl],
                                         start=False, stop=True)
                        nc.scalar.activation(gsb[:, wi, c, :], gp[:],
                                             Act.Copy)
                eng = nc.sync if blk % 2 == 0 else nc.gpsimd
                eng.dma_start(
                    gram_dram[2 * blk:2 * blk + 2].rearrange(
                        "w p c e -> p w (c e)"),
                    gsb[:].rearrange("p wi c e -> p wi (c e)"))

            emit_kv(0)
            for blk in range(1, 16):
                emit_kv(blk)
                emit_gram(blk - 1)
            emit_gram(15)

            # ---- combine: sel^T @ gram rows, quarter-pipelined; p-major
            # kvr_dram so the kvread loads are big contiguous descriptors.
            # q projections interleave with the combine's DMA-bound phase ----
            kvr_dram = dram_pool.tile([128, 32, 2, 129], bf)
            gflat = gram_dram[:].rearrange("w p c e -> w (p c e)")
            kvread = asb_pool.tile([128, 32, 2, 129], bf, tag="kvread", bufs=1)

            def emit_q(blk):
                tsl = slice(blk * 512, (blk + 1) * 512)
                for qc in range(2):
                    qp = mm512.tile([128, 512], f32, tag="mm512")
                    for c in range(2):
                        nc.tensor.matmul(qp[:], wq_sb[:, c, qc, :],
                                         xsb[:, c, tsl],
                                         start=(c == 0), stop=(c == 1))
                    nc.scalar.activation(qsb[:, qc, tsl], qp[:], Act.Sigmoid,
                                         bias=thqs_sb[:, qc:qc + 1], scale=1e6)

            def emit_combine_qtr(qtr):
                grow = grow_pool.tile([128, 2064], bf, tag="grow", bufs=2)
                for j in range(4):
                    jsl = slice(qtr * 8256 + j * 2064,
                                qtr * 8256 + (j + 1) * 2064)
                    eng = nc.gpsimd if j % 2 == 0 else nc.sync
                    eng.dma_start(grow[32 * j:32 * (j + 1), :],
                                  gflat[:, jsl])
                kvout = grow_pool.tile([128, 2064], bf, tag="kvout", bufs=2)
                for ch in range(6):
                    csl = slice(ch * CCH, (ch + 1) * CCH)
                    cp = mm512.tile([128, CCH], f32, tag="mm512")
                    for j in range(4):
                        nc.tensor.matmul(cp[32 * j:32 * (j + 1), :],
                                         selT_sb[32 * j:32 * (j + 1), :],
                                         grow[32 * j:32 * (j + 1), csl],
                                         start=True, stop=True,
                                         tile_position=(32 * j, 32 * j))
                    nc.vector.tensor_tensor(kvout[:, csl], cp[:],
                                            m64_sb[:, qtr, csl], op=mult)
                for j in range(4):
                    psl = slice(32 * qtr + 8 * j, 32 * qtr + 8 * j + 8)
                    eng = nc.gpsimd if j % 2 == 0 else nc.sync
                    eng.dma_start(
                        kvr_dram[psl].rearrange("p w c e -> w p (c e)"),
                        kvout[32 * j:32 * (j + 1), :].rearrange(
                            "w (p ce) -> w p ce", p=8))
                nc.gpsimd.dma_start(kvread[32 * qtr:32 * (qtr + 1)],
                                    kvr_dram[32 * qtr:32 * (qtr + 1)])

            for grp in range(4):
                for blk in range(4 * grp, 4 * grp + 4):
                    emit_q(blk)
                emit_combine_qtr(grp)

            # ---- den lhsT: ksum column replicated block-diagonally ----
            dexp = asb_pool.tile([128, 32, 2, 128], bf, tag="dexp", bufs=1)
            for c in range(2):
                nc.vector.tensor_tensor(
                    dexp[:, :, c, :],
                    kvread[:, :, c, 128:129].to_broadcast([128, 32, 128]),
                    bmask_sb[:, None, :].to_broadcast([128, 32, 128]),
                    op=mult)

            # ---- attention + den + divide; projection ----
            attn_nb = big_pool.tile([128, 2, NTOK], bf, tag="bigbuf", bufs=1)
            for blk in range(16):
                for wi in range(2):
                    w = blk * 2 + wi
                    wsl = slice(w * 256, (w + 1) * 256)
                    adpA = attnp.tile([128, 512], f32, tag="attn")
                    adpB = attnp.tile([128, 512], f32, tag="attn")
                    for c in range(2):
                        nc.tensor.matmul(adpA[:, 256 * c:256 * (c + 1)],
                                         kvread[:, w, c, 0:128],
                                         qsb[:, c, wsl],
                                         start=True, stop=True)
                        nc.tensor.matmul(adpB[:, 256 * c:256 * (c + 1)],
                                         dexp[:, w, c, :],
                                         qsb[:, c, wsl],
                                         start=True, stop=True)
                    den_sc = out_pool.tile([128, 512], f32, tag="densc",
                                           bufs=4)
                    nc.scalar.activation(den_sc[:], adpB[:], Act.Identity,
                                         bias=eps_sb[:, 0:1])
                    nc.vector.reciprocal_approx_fast(out=den_sc[:],
                                                     in_=den_sc[:])
                    nc.vector.tensor_tensor(
                        attn_nb[:, :, wsl],
                        adpA[:].rearrange("p (c s) -> p c s", c=2),
                        den_sc[:].rearrange("p (c s) -> p c s", c=2),
                        op=mult)
                tsl = slice(blk * 512, (blk + 1) * 512)
                osb = out_pool.tile([128, 2, 512], bf, tag="osb", bufs=3)
                for pc in range(2):
                    pjp = mm512.tile([128, 512], f32, tag="mm512")
                    for ec in range(2):
                        nc.tensor.matmul(pjp[:], wproj_sb[:, ec, pc, :],
                                         attn_nb[:, ec, tsl],
                                         start=(ec == 0), stop=(ec == 1))
                    nc.scalar.activation(osb[:, pc, :], pjp[:], Act.Identity,
                                         bias=bproj_sb[:, pc:pc + 1])
                eng = nc.sync if blk % 2 == 0 else nc.gpsimd
                eng.dma_start(outT[:, :, tsl], osb[:])

    nc.compile()
    return nc


def _prep_shared(w_qkv, b_qkv, w_proj, b_proj):
    wq_a = w_qkv[:, 0:256].reshape(2, 128, 2, 128).transpose(1, 0, 2, 3)
    th = 2.0 - b_qkv
    # fold the kv thresholds into the weights: (x@w >= th) == (x@(w/th) >= 1)
    # (th = 2 - b is positive for any remotely reasonable bias)
    thkv_col = th[256:768]
    assert (thkv_col > 0).all(), "kv thresholds must be positive"
    wkv_a = (w_qkv[:, 256:768] / thkv_col[None, :]).reshape(
        2, 128, 512).transpose(1, 0, 2)
    thq_a = th[0:256].reshape(2, 128).T
    thqs_a = -1e6 * thq_a
    thkv_a = np.ones((128, 512), np.float32)
    wproj_a = w_proj.reshape(2, 128, 2, 128).transpose(1, 0, 2, 3)
    bproj_a = b_proj.reshape(2, 128).T
    i = np.arange(128)[:, None]
    j = np.arange(128)[None, :]
    bmask_a = (i // 32) == (j // 32)
    # m64q[32*j + w, qtr, cw]: block-diag mask of flat gram col
    # flat = qtr*8256 + j*2064 + cw;  p = flat//258, e = (flat%258)%129
    rr = np.arange(128)[:, None, None]
    qq = np.arange(4)[None, :, None]
    cw = np.arange(2064)[None, None, :]
    flat = qq * 8256 + (rr // 32) * 2064 + cw
    pp_ = flat // 258
    ee = (flat % 258) % 129
    mask64q_a = ((pp_ // 32) == (ee // 32)) | (ee == 128)
    return {
        "wq": np.ascontiguousarray(wq_a).astype(BF16),
        "wkv": np.ascontiguousarray(wkv_a).astype(BF16),
        "thqs": np.ascontiguousarray(thqs_a).astype(np.float32),
        "thkv": np.ascontiguousarray(thkv_a).astype(np.float32),
        "wproj": np.ascontiguousarray(wproj_a).astype(BF16),
        "bproj": np.ascontiguousarray(bproj_a).astype(np.float32),
        "bmask": bmask_a.astype(BF16),
        "m64q": mask64q_a.astype(BF16),
    }


def window_partition(x):
    """[T,B,Lt,Lh,Lw,C] -> [T,B,NTOK,C] with tokens in (w, s) order."""
    Tb, Bb = x.shape[0], x.shape[1]
    xw = x.reshape(Tb, Bb, WT, LT, WH, LH, WW, LW, C)
    xw = xw.transpose(0, 1, 2, 4, 6, 3, 5, 7, 8)
    return np.ascontiguousarray(xw).reshape(Tb, Bb, NTOK, C)


def window_reverse(o):
    """[NTOK, C] -> [Lt, Lh, Lw, C]."""
    o = o.reshape(WT, WH, WW, LT, LH, LW, C)
    o = o.transpose(0, 3, 1, 4, 2, 5, 6)
    return np.ascontiguousarray(o).reshape(Lt, Lh, Lw, C)


def _routing_selT4(xw):
    """Host-side routing: region mean -> scores -> top-4 -> selT, replicated
    4x along partitions for the PE-quadrant combine. xw: [T,B,NTOK,C] f32."""
    xw5 = xw.reshape(T, B, W, S, C)
    region = xw5.mean(axis=(0, 3))                       # [B, W, C]
    scale = np.float32(D ** -0.5)
    scores = np.einsum('bwc,bvc->bwv', region, region) * scale
    idx = np.argsort(-scores, axis=-1, kind='stable')[:, :, :TOPK]
    sel = np.zeros((B, W, W), np.float32)
    b_ix = np.arange(B)[:, None, None]
    w_ix = np.arange(W)[None, :, None]
    sel[b_ix, w_ix, idx] = 1.0
    selT = sel.transpose(0, 2, 1)                        # [B, ws, wd]
    return np.tile(selT, (1, 4, 1)).astype(BF16)         # [B, 128, 32]


def run_kernel_spmd(nc, in_maps, **kwargs):
    from concourse.bass_utils import run_bass_kernel_spmd
    return run_bass_kernel_spmd(nc, in_maps, core_ids=list(range(NCORES)),
                                **kwargs)


def make_in_maps(x, w_qkv, b_qkv, w_proj, b_proj):
    x = np.asarray(x, dtype=np.float32)
    shared = _prep_shared(np.asarray(w_qkv, dtype=np.float32),
                          np.asarray(b_qkv, dtype=np.float32),
                          np.asarray(w_proj, dtype=np.float32),
                          np.asarray(b_proj, dtype=np.float32))
    xw = window_partition(x)
    selT4 = _routing_selT4(xw)
    in_maps = []
    for core in range(NCORES):
        b, t = core // 4, core % 4
        xt = np.ascontiguousarray(xw[t, b].T).astype(BF16)  # [C, NTOK]
        in_maps.append({**shared, "xT": xt.reshape(2, 128, NTOK),
                        "selT4": np.ascontiguousarray(selT4[b])})
    return in_maps


def unpack_out(res):
    out = np.empty((T, B, Lt, Lh, Lw, C), dtype=np.float32)
    for core in range(NCORES):
        b, t = core // 4, core % 4
        oT = np.asarray(res.results[core]["outT"],
                        dtype=np.float32).transpose(1, 0, 2).reshape(256, NTOK)
        out[t, b] = window_reverse(np.ascontiguousarray(oT.T))
    return out


def kernel(x, w_qkv, b_qkv, w_proj, b_proj):
    if "nc" not in _CACHE:
        _CACHE["nc"] = build_kernel()
    nc = _CACHE["nc"]
    in_maps = make_in_maps(x, w_qkv, b_qkv, w_proj, b_proj)
    res = run_kernel_spmd(nc, in_maps)
    return unpack_out(res)
